# revision 23
# baseline (speedup 1.0000x reference)
"""BitLinear (layernorm -> absmax sign-quant -> sign-weight matmul -> bias*beta)
for Trainium2, batch-sharded across 8 NeuronCores.

Math (per row b, feature i, output o):
    mean_b  = mean(x[b,:]);  var_b = var(x[b,:])
    xn      = (x - mean_b) * rsqrt(var_b + eps)
    c_b     = max_i |xn[b,i]|  = rsqrt(var_b+eps) * max(max_i x - mean_b, mean_b - min_i x)
    A[b,i]  = sign(x[b,i] - mean_b) / gamma[i]        (sign(xn) == sign(x - mean))
    out[b,o]= (c_b * sum_i A[b,i]*sign(W[o,i]) + bias[o]) * beta[o]

The +-1 sign operands are exact in bf16, and the fp32 PSUM accumulation of
+-1 products is exact, so the big GEMM runs on the TensorEngine in bf16 with
no accuracy loss. Each core handles 1024 batch rows; there are no collectives.
The host passes x, x^T and W^T (layout-only transforms) and transposes the
per-core [OUT, b] device output back to [b, OUT].
"""
import sys

sys.path.insert(0, "/opt/trn_rl_repo")

from contextlib import ExitStack

import numpy as np

import concourse.bass as bass
import concourse.tile as tile
from concourse import mybir
from concourse.bass_utils import run_bass_kernel_spmd
from concourse.vector_clock import ScopedClock, VectorClock

N_CORES = 8
EPS = 1e-5
P = 128


# ---------------------------------------------------------------------------
# Workaround: this walrus build rejects CTRL instructions (Drain/NoOp) with
# more than one sync wait. Tile's final drain carries one wait per live
# processor. Split them across single-wait SP nops; SP program order makes
# this equivalent.
def _patched_drain_and_barrier(self, tick_clock, wait_clock):
    gc = tick_clock.global_clock
    for scope, vclock in ScopedClock({None: gc}).items():
        n = len(vclock)
        for i in range(n):
            if vclock[i] > 0:
                vec = [0] * n
                vec[i] = vclock[i]
                nop_inst = self.nc.sync.nop(nofuse=True, hint="split_drain_wait")
                wait_clock.add_sem_waits(
                    nop_inst.ins, ScopedClock({scope: VectorClock(vec)})
                )
    self.nc.sync.drain()
    self.nc.all_engine_barrier()
    assert self.sems is not None
    popped = self.nc._tile_sem_poison_stack.pop()
    assert popped is self._sem_poison
    self.nc.clear_and_free_semaphores(list(self.sems.allocated().values()))
    self.nc.all_engine_barrier()


tile.TileContext._drain_and_barrier = _patched_drain_and_barrier


# This walrus build allows at most ONE sync wait on ANY instruction. Tile's
# wait-assignment emits up to 4. Post-process the serialized BIR: move all but
# the last wait of each instruction onto same-engine NoOps placed just before
# it (engine program order preserves semantics; for DMAs this gates descriptor
# submission, which is strictly more conservative).
def _split_multi_waits(m: dict) -> dict:
    for fn in m["functions"]:
        for bb in fn["blocks"]:
            out = []
            for ins in bb["instructions"]:
                si = ins.get("sync_info") or {}
                waits = si.get("on_wait") or []
                if len(waits) > 1:
                    for i, w in enumerate(waits[:-1]):
                        out.append(
                            {
                                "debug": ins.get("debug", 0),
                                "engine": ins["engine"],
                                "ins": [],
                                "outs": [],
                                "name": f"{ins['name']}-w{i}",
                                "opcode": "NoOp",
                                "sync_info": {"on_update": [], "on_wait": [w]},
                                "text_hint": "split_wait",
                            }
                        )
                    si["on_wait"] = [waits[-1]]
                out.append(ins)
            bb["instructions"] = out
    return m


_orig_to_json_bytes = bass.Bass.to_json_bytes


def _patched_to_json_bytes(self):
    import orjson

    m = orjson.loads(_orig_to_json_bytes(self))
    return orjson.dumps(_split_multi_waits(m))


bass.Bass.to_json_bytes = _patched_to_json_bytes
# ---------------------------------------------------------------------------


def build_bitlinear_program(b_c, d_in, d_out, apply_invgamma=True, use_fp8=True):
    """Bass program for one core: b_c batch rows, full d_in/d_out.

    use_fp8: run the sign matmul in fp8e4 with DoubleRow (2 MACs/cell/cycle).
    +-1 is exact in fp8e4, so this is lossless; only valid when invgamma is
    not folded into the signs (gamma == 1), else bf16 is used.
    """
    KT = d_in // P  # contraction tiles
    OG = d_out // P  # output-feature tiles
    BT = b_c // P  # batch tiles (stats stage)
    NB = 512  # matmul moving free dim = one PSUM bank of fp32
    BC = b_c // NB  # batch chunks in the matmul/epilogue
    SC = min(512, d_in)  # bn_stats hardware max free size
    nstat = d_in // SC
    if use_fp8:
        assert not apply_invgamma and KT % 2 == 0

    f32 = mybir.dt.float32
    bf16 = mybir.dt.bfloat16
    fp8 = mybir.dt.float8e4
    sdt = fp8 if use_fp8 else bf16  # sign dtype
    wdt = bf16  # weight transfer dtype (sign-preserving host cast)
    X = mybir.AxisListType.X
    A = mybir.AluOpType
    AF = mybir.ActivationFunctionType

    G = min(4, KT)  # k-tiles per transposed-input DMA (8 KiB runs/partition)

    nc = bass.Bass("TRN2", target_bir_lowering=False, debug=False)
    x = nc.dram_tensor("x", [b_c, d_in], f32, kind="ExternalInput")
    # host-prechunked transpose: xTc[h, p, kt, j] = x[h*NB + j, kt*128 + p]
    xTc = nc.dram_tensor("xTc", [BC, P, KT, NB], f32, kind="ExternalInput")
    # host-pretiled weight signsource: w4[og, p, kt, oc] = W[og*128+oc, kt*128+p]
    # so each per-og DMA is one contiguous run per partition. Transferred as
    # bf16 (round-to-nearest preserves the sign of every nonzero float).
    w4 = nc.dram_tensor("w4", [OG, P, KT, P], wdt, kind="ExternalInput")
    bias = nc.dram_tensor("bias", [d_out], f32, kind="ExternalInput")
    beta = nc.dram_tensor("beta", [d_out], f32, kind="ExternalInput")
    gamma = nc.dram_tensor("gamma", [d_in], f32, kind="ExternalInput")
    outT = nc.dram_tensor("outT", [d_out, b_c], f32, kind="ExternalOutput")
    # per-batch-chunk scratches so a chunk's broadcast only depends on that
    # chunk's stats tiles (Tile tracks DRAM deps per tensor).
    mean_ds = [nc.dram_tensor(f"mean_d{h}", [NB], f32) for h in range(BC)]
    c_ds = [nc.dram_tensor(f"c_d{h}", [NB], f32) for h in range(BC)]

    with tile.TileContext(nc) as tc, ExitStack() as ctx:
        consts = ctx.enter_context(tc.tile_pool(name="consts", bufs=1))
        stats_p = ctx.enter_context(tc.tile_pool(name="stats", bufs=NB // P))
        small_p = ctx.enter_context(tc.tile_pool(name="small", bufs=4))
        a_p = ctx.enter_context(tc.tile_pool(name="a", bufs=1))
        xt_p = ctx.enter_context(tc.tile_pool(name="xt", bufs=2))
        w_p = ctx.enter_context(tc.tile_pool(name="w", bufs=2))
        sw_p = ctx.enter_context(tc.tile_pool(name="sw", bufs=6))
        ep_p = ctx.enter_context(tc.tile_pool(name="ep", bufs=2))
        ps_p = ctx.enter_context(tc.tile_pool(name="ps", bufs=2 * BC, space="PSUM"))

        # --- constants ---------------------------------------------------
        eps_t = consts.tile([P, 1], f32)
        nc.vector.memset(eps_t, EPS)
        # column j of these holds v[j*128 : (j+1)*128] (per-partition scalars)
        bias_t = consts.tile([P, OG], f32)
        nc.sync.dma_start(
            out=bias_t, in_=bass.AP(tensor=bias, offset=0, ap=[[1, P], [P, OG]])
        )
        beta_t = consts.tile([P, OG], f32)
        nc.sync.dma_start(
            out=beta_t, in_=bass.AP(tensor=beta, offset=0, ap=[[1, P], [P, OG]])
        )
        bb_t = consts.tile([P, OG], f32)
        nc.vector.tensor_mul(bb_t, bias_t, beta_t)
        if apply_invgamma:
            gamma_t = consts.tile([P, KT], f32)
            nc.sync.dma_start(
                out=gamma_t, in_=bass.AP(tensor=gamma, offset=0, ap=[[1, P], [P, KT]])
            )
            invg = consts.tile([P, KT], f32)
            nc.vector.reciprocal(invg, gamma_t)

        # --- prologue: sign the first few weight columns while the input is
        # still streaming in, so the PE has work the moment the first input
        # signs land (program order == scheduler priority).
        PREW = min(4, OG)
        pre_sw = {}
        for og in range(PREW):
            wcol = w_p.tile([P, KT * P], wdt, tag="wcol")
            nc.sync.dma_start(
                out=wcol,
                in_=bass.AP(
                    tensor=w4, offset=og * P * KT * P, ap=[[KT * P, P], [1, KT * P]]
                ),
            )
            sw = sw_p.tile([P, KT, P], sdt, tag="sw", name=f"swpre{og}")
            nc.scalar.sign(out=sw, in_=wcol.rearrange("p (kt oc) -> p kt oc", oc=P))
            pre_sw[og] = sw

        # --- stages 1-3, chunked by batch chunk (NB rows) so the first
        # chunk's signs (and therefore the matmuls) start after only that
        # chunk's stats. The mean is computed on ScalarE via activation
        # accumulate (fp32 accumulator) so the DVE never sits on the
        # mean->sign critical path; max/min/var work is deferred to after the
        # chunk's signs (it only feeds the epilogue scale c).
        a_t = a_p.tile([P, KT, b_c], sdt)
        junk = consts.tile([P, d_in], bf16)  # ACT accumulate writes land here
        dsc = consts.tile([P, d_in], f32)  # centered-x scratch (stats)
        mean_bs = []
        cbs = []
        TPC = NB // P  # stats tiles per chunk
        inv_n = 1.0 / d_in
        for h in range(BC):
            x_nats = []
            means = []
            # fast path to the chunk's mean: ACT sum + tiny scale
            for bth in range(TPC):
                bt = h * TPC + bth
                x_nat = stats_p.tile([P, d_in], f32, tag="xnat", name=f"xn{bth}")
                # split across 4 DMA queues — one queue moves only ~50 GB/s
                QS = d_in // 4
                for q in range(4):
                    nc.sync.dma_start(
                        out=x_nat[:, q * QS : (q + 1) * QS],
                        in_=x[bt * P : (bt + 1) * P, q * QS : (q + 1) * QS],
                    )
                x_nats.append(x_nat)
                s_sum = small_p.tile([P, 1], f32, tag="ssum")
                nc.scalar.activation(
                    out=junk, in_=x_nat, func=AF.Copy, accum_out=s_sum
                )
                mean = small_p.tile([P, 1], f32, tag="mean", name=f"mean{bth}")
                nc.scalar.mul(out=mean, in_=s_sum, mul=inv_n)
                means.append(mean)
                nc.sync.dma_start(out=mean_ds[h][bth * P : (bth + 1) * P], in_=mean)

            # broadcast this chunk's mean across partitions
            mean_b = consts.tile([P, NB], f32, name=f"mean_b{h}")
            nc.sync.dma_start(
                out=mean_b,
                in_=bass.AP(tensor=mean_ds[h], offset=0, ap=[[0, P], [1, NB]]),
            )
            mean_bs.append(mean_b)

            # signs for this chunk, all k tiles (loaded G k-tiles per DMA)
            for gi in range(KT // G):
                xtg = xt_p.tile([P, G, NB], f32, tag="xtg")
                nc.sync.dma_start(
                    out=xtg,
                    in_=bass.AP(
                        tensor=xTc,
                        offset=h * P * KT * NB + gi * G * NB,
                        ap=[[KT * NB, P], [1, G * NB]],
                    ),
                )
                for r in range(G):
                    kt = gi * G + r
                    nc.vector.tensor_sub(xtg[:, r, :], xtg[:, r, :], mean_b)
                    dst = a_t[:, kt, h * NB : (h + 1) * NB]
                    if apply_invgamma:
                        stmp = xt_p.tile([P, NB], bf16, tag="stmp")
                        nc.scalar.sign(out=stmp, in_=xtg[:, r, :])
                        nc.vector.tensor_scalar_mul(
                            out=dst, in0=stmp, scalar1=invg[:, kt : kt + 1]
                        )
                    else:
                        nc.scalar.sign(out=dst, in_=xtg[:, r, :])

            # deferred: c = max|x - mean| / sqrt(var + eps), one centered pass
            for bth in range(TPC):
                x_nat = x_nats[bth]
                mean = means[bth]
                nc.vector.tensor_scalar(
                    out=dsc,
                    in0=x_nat,
                    scalar1=mean,
                    scalar2=None,
                    op0=A.subtract,
                )
                amax = small_p.tile([P, 1], f32, tag="amax")
                nc.vector.tensor_reduce(
                    out=amax, in_=dsc, axis=X, op=A.max, apply_absolute_value=True
                )
                ssq = small_p.tile([P, 1], f32, tag="ssq")
                nc.scalar.activation(
                    out=junk, in_=dsc, func=AF.Square, accum_out=ssq
                )
                # c = amax / sqrt(sum(d^2)/n + eps)
                var = small_p.tile([P, 1], f32, tag="var")
                nc.scalar.mul(out=var, in_=ssq, mul=inv_n)
                std = small_p.tile([P, 1], f32, tag="std")
                nc.scalar.activation(out=std, in_=var, func=AF.Sqrt, bias=eps_t)
                rstd = small_p.tile([P, 1], f32, tag="rstd")
                nc.vector.reciprocal(rstd, std)
                cv = small_p.tile([P, 1], f32, tag="cv")
                nc.vector.tensor_mul(cv, amax, rstd)
                nc.sync.dma_start(out=c_ds[h][bth * P : (bth + 1) * P], in_=cv)

            cb = consts.tile([P, NB], f32, name=f"cb{h}")
            nc.sync.dma_start(
                out=cb, in_=bass.AP(tensor=c_ds[h], offset=0, ap=[[0, P], [1, NB]])
            )
            cbs.append(cb)

        # --- stage 4: weight signs + matmul + epilogue --------------------
        for og in range(OG):
            if og in pre_sw:
                sw = pre_sw[og]
            else:
                wcol = w_p.tile([P, KT * P], wdt, tag="wcol")
                nc.sync.dma_start(
                    out=wcol,
                    in_=bass.AP(
                        tensor=w4,
                        offset=og * P * KT * P,
                        ap=[[KT * P, P], [1, KT * P]],
                    ),
                )
                wcol3 = wcol.rearrange("p (kt oc) -> p kt oc", oc=P)
                sw = sw_p.tile([P, KT, P], sdt, tag="sw")
                nc.scalar.sign(out=sw, in_=wcol3)
            psums = [
                ps_p.tile([P, NB], f32, tag=f"ps{bc}", name=f"psum{bc}")
                for bc in range(BC)
            ]
            if use_fp8:
                for g in range(KT // 2):
                    for bc in range(BC):
                        nc.tensor.matmul(
                            psums[bc],
                            lhsT=sw[:, 2 * g : 2 * g + 2, :],
                            rhs=a_t[:, 2 * g : 2 * g + 2, bc * NB : (bc + 1) * NB],
                            start=(g == 0),
                            stop=(g == KT // 2 - 1),
                            perf_mode=mybir.MatmulPerfMode.DoubleRow,
                        )
            else:
                for kt in range(KT):
                    for bc in range(BC):
                        nc.tensor.matmul(
                            psums[bc],
                            lhsT=sw[:, kt, :],
                            rhs=a_t[:, kt, bc * NB : (bc + 1) * NB],
                            start=(kt == 0),
                            stop=(kt == KT - 1),
                        )
            o_sb = ep_p.tile([P, b_c], f32, tag="osb")
            for bc in range(BC):
                t1 = ep_p.tile([P, NB], f32, tag="t1")
                nc.vector.tensor_tensor(
                    out=t1, in0=psums[bc], in1=cbs[bc], op=A.mult
                )
                # out = t1*beta + bias*beta, per-partition scalars, on DVE
                nc.vector.tensor_scalar(
                    out=o_sb[:, bc * NB : (bc + 1) * NB],
                    in0=t1,
                    scalar1=beta_t[:, og : og + 1],
                    scalar2=bb_t[:, og : og + 1],
                    op0=A.mult,
                    op1=A.add,
                )
            nc.sync.dma_start(out=outT[og * P : (og + 1) * P, :], in_=o_sb)

    return nc


def kernel(input, weight, bias, gamma, beta, _run_kwargs=None):
    input = np.ascontiguousarray(np.asarray(input, dtype=np.float32))
    weight = np.ascontiguousarray(np.asarray(weight, dtype=np.float32))
    bias = np.ascontiguousarray(np.asarray(bias, dtype=np.float32))
    gamma = np.ascontiguousarray(np.asarray(gamma, dtype=np.float32))
    beta = np.ascontiguousarray(np.asarray(beta, dtype=np.float32))

    B, d_in = input.shape
    d_out = weight.shape[0]
    assert B % N_CORES == 0
    b_c = B // N_CORES

    apply_invgamma = not bool(np.all(gamma == 1.0))
    use_fp8 = not apply_invgamma
    nc = build_bitlinear_program(
        b_c, d_in, d_out, apply_invgamma=apply_invgamma, use_fp8=use_fp8
    )

    # w4[og, p, kt, oc] = weight[og*128+oc, kt*128+p] — layout transform so
    # each per-og weight DMA is one contiguous run per partition. Shipped as
    # bf16: round-to-nearest preserves the sign of every nonzero float, and
    # the device only takes sign(w).
    import ml_dtypes

    OG, KT = d_out // 128, d_in // 128
    w4 = np.ascontiguousarray(
        weight.reshape(OG, 128, KT, 128).transpose(0, 3, 2, 1)
    ).astype(ml_dtypes.bfloat16)

    NB = 512
    BC = b_c // NB
    in_maps = []
    for c in range(N_CORES):
        sl = slice(c * b_c, (c + 1) * b_c)
        x_c = np.ascontiguousarray(input[sl, :])
        # xTc[h, p, kt, j] = x_c[h*NB + j, kt*128 + p]
        xTc = np.ascontiguousarray(
            x_c.reshape(BC, NB, KT, 128).transpose(0, 3, 2, 1)
        )
        in_maps.append(
            {
                "x": x_c,
                "xTc": xTc,
                "w4": w4,
                "bias": bias,
                "beta": beta,
                "gamma": gamma,
            }
        )

    res = run_bass_kernel_spmd(
        nc, in_maps, core_ids=list(range(N_CORES)), **(_run_kwargs or {})
    )

    out = np.empty((B, d_out), dtype=np.float32)
    for c in range(N_CORES):
        out[c * b_c : (c + 1) * b_c, :] = res.results[c]["outT"].T
    if _run_kwargs:
        kernel.last_results = res
    return out


# revision 27
# speedup vs baseline: 1.0279x; 1.0279x over previous
"""BitLinear (layernorm -> absmax sign-quant -> sign-weight matmul -> bias*beta)
for Trainium2, batch-sharded across 8 NeuronCores.

Math (per row b, feature i, output o):
    mean_b  = mean(x[b,:]);  var_b = var(x[b,:])
    xn      = (x - mean_b) * rsqrt(var_b + eps)
    c_b     = max_i |xn[b,i]|  = rsqrt(var_b+eps) * max(max_i x - mean_b, mean_b - min_i x)
    A[b,i]  = sign(x[b,i] - mean_b) / gamma[i]        (sign(xn) == sign(x - mean))
    out[b,o]= (c_b * sum_i A[b,i]*sign(W[o,i]) + bias[o]) * beta[o]

The +-1 sign operands are exact in bf16, and the fp32 PSUM accumulation of
+-1 products is exact, so the big GEMM runs on the TensorEngine in bf16 with
no accuracy loss. Each core handles 1024 batch rows; there are no collectives.
The host passes x, x^T and W^T (layout-only transforms) and transposes the
per-core [OUT, b] device output back to [b, OUT].
"""
import sys

sys.path.insert(0, "/opt/trn_rl_repo")

from contextlib import ExitStack

import numpy as np

import concourse.bass as bass
import concourse.tile as tile
from concourse import mybir
from concourse.bass_utils import run_bass_kernel_spmd
from concourse.vector_clock import ScopedClock, VectorClock

N_CORES = 8
EPS = 1e-5
P = 128


# ---------------------------------------------------------------------------
# Workaround: this walrus build rejects CTRL instructions (Drain/NoOp) with
# more than one sync wait. Tile's final drain carries one wait per live
# processor. Split them across single-wait SP nops; SP program order makes
# this equivalent.
def _patched_drain_and_barrier(self, tick_clock, wait_clock):
    gc = tick_clock.global_clock
    for scope, vclock in ScopedClock({None: gc}).items():
        n = len(vclock)
        for i in range(n):
            if vclock[i] > 0:
                vec = [0] * n
                vec[i] = vclock[i]
                nop_inst = self.nc.sync.nop(nofuse=True, hint="split_drain_wait")
                wait_clock.add_sem_waits(
                    nop_inst.ins, ScopedClock({scope: VectorClock(vec)})
                )
    self.nc.sync.drain()
    self.nc.all_engine_barrier()
    assert self.sems is not None
    popped = self.nc._tile_sem_poison_stack.pop()
    assert popped is self._sem_poison
    self.nc.clear_and_free_semaphores(list(self.sems.allocated().values()))
    self.nc.all_engine_barrier()


tile.TileContext._drain_and_barrier = _patched_drain_and_barrier


# This walrus build allows at most ONE sync wait on ANY instruction. Tile's
# wait-assignment emits up to 4. Post-process the serialized BIR: move all but
# the last wait of each instruction onto same-engine NoOps placed just before
# it (engine program order preserves semantics; for DMAs this gates descriptor
# submission, which is strictly more conservative).
def _split_multi_waits(m: dict) -> dict:
    for fn in m["functions"]:
        for bb in fn["blocks"]:
            out = []
            for ins in bb["instructions"]:
                si = ins.get("sync_info") or {}
                waits = si.get("on_wait") or []
                if len(waits) > 1:
                    for i, w in enumerate(waits[:-1]):
                        out.append(
                            {
                                "debug": ins.get("debug", 0),
                                "engine": ins["engine"],
                                "ins": [],
                                "outs": [],
                                "name": f"{ins['name']}-w{i}",
                                "opcode": "NoOp",
                                "sync_info": {"on_update": [], "on_wait": [w]},
                                "text_hint": "split_wait",
                            }
                        )
                    si["on_wait"] = [waits[-1]]
                out.append(ins)
            bb["instructions"] = out
    return m


_orig_to_json_bytes = bass.Bass.to_json_bytes


def _patched_to_json_bytes(self):
    import orjson

    m = orjson.loads(_orig_to_json_bytes(self))
    return orjson.dumps(_split_multi_waits(m))


bass.Bass.to_json_bytes = _patched_to_json_bytes
# ---------------------------------------------------------------------------


def build_bitlinear_program(b_c, d_in, d_out, apply_invgamma=True, use_fp8=True):
    """Bass program for one core: b_c batch rows, full d_in/d_out.

    use_fp8: run the sign matmul in fp8e4 with DoubleRow (2 MACs/cell/cycle).
    +-1 is exact in fp8e4, so this is lossless; only valid when invgamma is
    not folded into the signs (gamma == 1), else bf16 is used.
    """
    KT = d_in // P  # contraction tiles
    OG = d_out // P  # output-feature tiles
    BT = b_c // P  # batch tiles (stats stage)
    NB = 512  # matmul moving free dim = one PSUM bank of fp32
    BC = b_c // NB  # batch chunks in the matmul/epilogue
    SC = min(512, d_in)  # bn_stats hardware max free size
    nstat = d_in // SC
    if use_fp8:
        assert not apply_invgamma and KT % 2 == 0

    f32 = mybir.dt.float32
    bf16 = mybir.dt.bfloat16
    fp8 = mybir.dt.float8e4
    sdt = fp8 if use_fp8 else bf16  # sign dtype
    wdt = bf16  # weight transfer dtype (sign-preserving host cast)
    X = mybir.AxisListType.X
    A = mybir.AluOpType
    AF = mybir.ActivationFunctionType

    G = min(4, KT)  # k-tiles per transposed-input DMA (8 KiB runs/partition)

    nc = bass.Bass("TRN2", target_bir_lowering=False, debug=False)
    x = nc.dram_tensor("x", [b_c, d_in], f32, kind="ExternalInput")
    # host-prechunked transpose: xTc[h, p, kt, j] = x[h*NB + j, kt*128 + p]
    xTc = nc.dram_tensor("xTc", [BC, P, KT, NB], f32, kind="ExternalInput")
    # host-pretiled weight signsource: w4[og, p, kt, oc] = W[og*128+oc, kt*128+p]
    # so each per-og DMA is one contiguous run per partition. Transferred as
    # bf16 (round-to-nearest preserves the sign of every nonzero float).
    w4 = nc.dram_tensor("w4", [OG, P, KT, P], wdt, kind="ExternalInput")
    bias = nc.dram_tensor("bias", [d_out], f32, kind="ExternalInput")
    beta = nc.dram_tensor("beta", [d_out], f32, kind="ExternalInput")
    gamma = nc.dram_tensor("gamma", [d_in], f32, kind="ExternalInput")
    outT = nc.dram_tensor("outT", [d_out, b_c], f32, kind="ExternalOutput")
    # per-batch-chunk scratches so a chunk's broadcast only depends on that
    # chunk's stats tiles (Tile tracks DRAM deps per tensor).
    mean_ds = [nc.dram_tensor(f"mean_d{h}", [NB], f32) for h in range(BC)]
    c_ds = [nc.dram_tensor(f"c_d{h}", [NB], f32) for h in range(BC)]

    with tile.TileContext(nc) as tc, ExitStack() as ctx:
        consts = ctx.enter_context(tc.tile_pool(name="consts", bufs=1))
        stats_p = ctx.enter_context(tc.tile_pool(name="stats", bufs=NB // P))
        small_p = ctx.enter_context(tc.tile_pool(name="small", bufs=4))
        a_p = ctx.enter_context(tc.tile_pool(name="a", bufs=1))
        xt_p = ctx.enter_context(tc.tile_pool(name="xt", bufs=2))
        w_p = ctx.enter_context(tc.tile_pool(name="w", bufs=2))
        sw_p = ctx.enter_context(tc.tile_pool(name="sw", bufs=5))
        ep_p = ctx.enter_context(tc.tile_pool(name="ep", bufs=4))
        ps_p = ctx.enter_context(tc.tile_pool(name="ps", bufs=2 * BC, space="PSUM"))

        # --- constants ---------------------------------------------------
        eps_t = consts.tile([P, 1], f32)
        nc.vector.memset(eps_t, EPS)
        # column j of these holds v[j*128 : (j+1)*128] (per-partition scalars)
        bias_t = consts.tile([P, OG], f32)
        nc.sync.dma_start(
            out=bias_t, in_=bass.AP(tensor=bias, offset=0, ap=[[1, P], [P, OG]])
        )
        beta_t = consts.tile([P, OG], f32)
        nc.sync.dma_start(
            out=beta_t, in_=bass.AP(tensor=beta, offset=0, ap=[[1, P], [P, OG]])
        )
        bb_t = consts.tile([P, OG], f32)
        nc.vector.tensor_mul(bb_t, bias_t, beta_t)
        if apply_invgamma:
            gamma_t = consts.tile([P, KT], f32)
            nc.sync.dma_start(
                out=gamma_t, in_=bass.AP(tensor=gamma, offset=0, ap=[[1, P], [P, KT]])
            )
            invg = consts.tile([P, KT], f32)
            nc.vector.reciprocal(invg, gamma_t)

        # --- prologue: issue chunk-0's input loads before anything else so
        # they own the DMA queue heads (HBM is ~360 GB/s/core and the early
        # window is oversubscribed), then sign the first weight columns while
        # those loads stream, so the PE has work the moment input signs land.
        TPC0 = NB // P
        QS = d_in // 4
        x_nat0 = []
        for bth in range(TPC0):
            x_nat = stats_p.tile([P, d_in], f32, tag="xnat", name=f"xn{bth}")
            for q in range(4):
                nc.sync.dma_start(
                    out=x_nat[:, q * QS : (q + 1) * QS],
                    in_=x[bth * P : (bth + 1) * P, q * QS : (q + 1) * QS],
                )
            x_nat0.append(x_nat)

        PREW = min(4, OG)
        pre_sw = {}
        for og in range(PREW):
            wcol = w_p.tile([P, KT * P], wdt, tag="wcol")
            nc.sync.dma_start(
                out=wcol,
                in_=bass.AP(
                    tensor=w4, offset=og * P * KT * P, ap=[[KT * P, P], [1, KT * P]]
                ),
            )
            sw = sw_p.tile([P, KT, P], sdt, tag="sw", name=f"swpre{og}")
            nc.scalar.sign(out=sw, in_=wcol.rearrange("p (kt oc) -> p kt oc", oc=P))
            pre_sw[og] = sw

        # --- stages 1-3, chunked by batch chunk (NB rows) so the first
        # chunk's signs (and therefore the matmuls) start after only that
        # chunk's stats. The mean is computed on ScalarE via activation
        # accumulate (fp32 accumulator) so the DVE never sits on the
        # mean->sign critical path; max/min/var work is deferred to after the
        # chunk's signs (it only feeds the epilogue scale c).
        a_t = a_p.tile([P, KT, b_c], sdt)
        junk = consts.tile([P, d_in], bf16)  # ACT accumulate writes land here
        dsc = consts.tile([P, d_in], f32)  # centered-x scratch (stats)
        mean_bs = []
        cbs = []
        TPC = NB // P  # stats tiles per chunk
        inv_n = 1.0 / d_in
        for h in range(BC):
            x_nats = []
            means = []
            # fast path to the chunk's mean: ACT sum + tiny scale
            for bth in range(TPC):
                bt = h * TPC + bth
                if h == 0:
                    x_nat = x_nat0[bth]  # loaded in the prologue
                else:
                    x_nat = stats_p.tile([P, d_in], f32, tag="xnat", name=f"xn{bth}")
                    for q in range(4):
                        nc.sync.dma_start(
                            out=x_nat[:, q * QS : (q + 1) * QS],
                            in_=x[bt * P : (bt + 1) * P, q * QS : (q + 1) * QS],
                        )
                x_nats.append(x_nat)
                s_sum = small_p.tile([P, 1], f32, tag="ssum")
                nc.scalar.activation(
                    out=junk, in_=x_nat, func=AF.Copy, accum_out=s_sum
                )
                mean = small_p.tile([P, 1], f32, tag="mean", name=f"mean{bth}")
                nc.scalar.mul(out=mean, in_=s_sum, mul=inv_n)
                means.append(mean)
                nc.sync.dma_start(out=mean_ds[h][bth * P : (bth + 1) * P], in_=mean)

            # broadcast this chunk's mean across partitions
            mean_b = consts.tile([P, NB], f32, name=f"mean_b{h}")
            nc.sync.dma_start(
                out=mean_b,
                in_=bass.AP(tensor=mean_ds[h], offset=0, ap=[[0, P], [1, NB]]),
            )
            mean_bs.append(mean_b)

            # signs for this chunk, all k tiles (loaded G k-tiles per DMA)
            for gi in range(KT // G):
                xtg = xt_p.tile([P, G, NB], f32, tag="xtg")
                nc.sync.dma_start(
                    out=xtg,
                    in_=bass.AP(
                        tensor=xTc,
                        offset=h * P * KT * NB + gi * G * NB,
                        ap=[[KT * NB, P], [1, G * NB]],
                    ),
                )
                for r in range(G):
                    kt = gi * G + r
                    nc.vector.tensor_sub(xtg[:, r, :], xtg[:, r, :], mean_b)
                    dst = a_t[:, kt, h * NB : (h + 1) * NB]
                    if apply_invgamma:
                        stmp = xt_p.tile([P, NB], bf16, tag="stmp")
                        nc.scalar.sign(out=stmp, in_=xtg[:, r, :])
                        nc.vector.tensor_scalar_mul(
                            out=dst, in0=stmp, scalar1=invg[:, kt : kt + 1]
                        )
                    else:
                        nc.scalar.sign(out=dst, in_=xtg[:, r, :])

            # deferred: c = max|x - mean| / sqrt(var + eps), one centered pass
            for bth in range(TPC):
                x_nat = x_nats[bth]
                mean = means[bth]
                nc.vector.tensor_scalar(
                    out=dsc,
                    in0=x_nat,
                    scalar1=mean,
                    scalar2=None,
                    op0=A.subtract,
                )
                amax = small_p.tile([P, 1], f32, tag="amax")
                nc.vector.tensor_reduce(
                    out=amax, in_=dsc, axis=X, op=A.max, apply_absolute_value=True
                )
                ssq = small_p.tile([P, 1], f32, tag="ssq")
                nc.scalar.activation(
                    out=junk, in_=dsc, func=AF.Square, accum_out=ssq
                )
                # c = amax / sqrt(sum(d^2)/n + eps)
                var = small_p.tile([P, 1], f32, tag="var")
                nc.scalar.mul(out=var, in_=ssq, mul=inv_n)
                std = small_p.tile([P, 1], f32, tag="std")
                nc.scalar.activation(out=std, in_=var, func=AF.Sqrt, bias=eps_t)
                rstd = small_p.tile([P, 1], f32, tag="rstd")
                nc.vector.reciprocal(rstd, std)
                cv = small_p.tile([P, 1], f32, tag="cv")
                nc.vector.tensor_mul(cv, amax, rstd)
                nc.sync.dma_start(out=c_ds[h][bth * P : (bth + 1) * P], in_=cv)

            cb = consts.tile([P, NB], f32, name=f"cb{h}")
            nc.sync.dma_start(
                out=cb, in_=bass.AP(tensor=c_ds[h], offset=0, ap=[[0, P], [1, NB]])
            )
            cbs.append(cb)

        # --- stage 4: weight signs + matmul + epilogue --------------------
        for og in range(OG):
            if og in pre_sw:
                sw = pre_sw[og]
            else:
                wcol = w_p.tile([P, KT * P], wdt, tag="wcol")
                nc.sync.dma_start(
                    out=wcol,
                    in_=bass.AP(
                        tensor=w4,
                        offset=og * P * KT * P,
                        ap=[[KT * P, P], [1, KT * P]],
                    ),
                )
                wcol3 = wcol.rearrange("p (kt oc) -> p kt oc", oc=P)
                sw = sw_p.tile([P, KT, P], sdt, tag="sw")
                nc.scalar.sign(out=sw, in_=wcol3)
            psums = [
                ps_p.tile([P, NB], f32, tag=f"ps{bc}", name=f"psum{bc}")
                for bc in range(BC)
            ]
            if use_fp8:
                for g in range(KT // 2):
                    for bc in range(BC):
                        nc.tensor.matmul(
                            psums[bc],
                            lhsT=sw[:, 2 * g : 2 * g + 2, :],
                            rhs=a_t[:, 2 * g : 2 * g + 2, bc * NB : (bc + 1) * NB],
                            start=(g == 0),
                            stop=(g == KT // 2 - 1),
                            perf_mode=mybir.MatmulPerfMode.DoubleRow,
                        )
            else:
                for kt in range(KT):
                    for bc in range(BC):
                        nc.tensor.matmul(
                            psums[bc],
                            lhsT=sw[:, kt, :],
                            rhs=a_t[:, kt, bc * NB : (bc + 1) * NB],
                            start=(kt == 0),
                            stop=(kt == KT - 1),
                        )
            for bc in range(BC):
                t1 = ep_p.tile([P, NB], f32, tag="t1")
                nc.vector.tensor_tensor(
                    out=t1, in0=psums[bc], in1=cbs[bc], op=A.mult
                )
                # out = t1*beta + bias*beta, per-partition scalars, on DVE
                o_sb = ep_p.tile([P, NB], f32, tag="osb")
                nc.vector.tensor_scalar(
                    out=o_sb,
                    in0=t1,
                    scalar1=beta_t[:, og : og + 1],
                    scalar2=bb_t[:, og : og + 1],
                    op0=A.mult,
                    op1=A.add,
                )
                nc.sync.dma_start(
                    out=outT[og * P : (og + 1) * P, bc * NB : (bc + 1) * NB],
                    in_=o_sb,
                )

    return nc


def kernel(input, weight, bias, gamma, beta, _run_kwargs=None):
    input = np.ascontiguousarray(np.asarray(input, dtype=np.float32))
    weight = np.ascontiguousarray(np.asarray(weight, dtype=np.float32))
    bias = np.ascontiguousarray(np.asarray(bias, dtype=np.float32))
    gamma = np.ascontiguousarray(np.asarray(gamma, dtype=np.float32))
    beta = np.ascontiguousarray(np.asarray(beta, dtype=np.float32))

    B, d_in = input.shape
    d_out = weight.shape[0]
    assert B % N_CORES == 0
    b_c = B // N_CORES

    apply_invgamma = not bool(np.all(gamma == 1.0))
    use_fp8 = not apply_invgamma
    nc = build_bitlinear_program(
        b_c, d_in, d_out, apply_invgamma=apply_invgamma, use_fp8=use_fp8
    )

    # w4[og, p, kt, oc] = weight[og*128+oc, kt*128+p] — layout transform so
    # each per-og weight DMA is one contiguous run per partition. Shipped as
    # bf16: round-to-nearest preserves the sign of every nonzero float, and
    # the device only takes sign(w).
    import ml_dtypes

    OG, KT = d_out // 128, d_in // 128
    w4 = np.ascontiguousarray(
        weight.reshape(OG, 128, KT, 128).transpose(0, 3, 2, 1)
    ).astype(ml_dtypes.bfloat16)

    NB = 512
    BC = b_c // NB
    in_maps = []
    for c in range(N_CORES):
        sl = slice(c * b_c, (c + 1) * b_c)
        x_c = np.ascontiguousarray(input[sl, :])
        # xTc[h, p, kt, j] = x_c[h*NB + j, kt*128 + p]
        xTc = np.ascontiguousarray(
            x_c.reshape(BC, NB, KT, 128).transpose(0, 3, 2, 1)
        )
        in_maps.append(
            {
                "x": x_c,
                "xTc": xTc,
                "w4": w4,
                "bias": bias,
                "beta": beta,
                "gamma": gamma,
            }
        )

    res = run_bass_kernel_spmd(
        nc, in_maps, core_ids=list(range(N_CORES)), **(_run_kwargs or {})
    )

    out = np.empty((B, d_out), dtype=np.float32)
    for c in range(N_CORES):
        out[c * b_c : (c + 1) * b_c, :] = res.results[c]["outT"].T
    if _run_kwargs:
        kernel.last_results = res
    return out


# revision 30
# speedup vs baseline: 1.0480x; 1.0196x over previous
"""BitLinear (layernorm -> absmax sign-quant -> sign-weight matmul -> bias*beta)
for Trainium2, batch-sharded across 8 NeuronCores.

Math (per row b, feature i, output o):
    mean_b  = mean(x[b,:]);  var_b = var(x[b,:])
    xn      = (x - mean_b) * rsqrt(var_b + eps)
    c_b     = max_i |xn[b,i]|  = rsqrt(var_b+eps) * max(max_i x - mean_b, mean_b - min_i x)
    A[b,i]  = sign(x[b,i] - mean_b) / gamma[i]        (sign(xn) == sign(x - mean))
    out[b,o]= (c_b * sum_i A[b,i]*sign(W[o,i]) + bias[o]) * beta[o]

The +-1 sign operands are exact in bf16, and the fp32 PSUM accumulation of
+-1 products is exact, so the big GEMM runs on the TensorEngine in bf16 with
no accuracy loss. Each core handles 1024 batch rows; there are no collectives.
The host passes x, x^T and W^T (layout-only transforms) and transposes the
per-core [OUT, b] device output back to [b, OUT].
"""
import sys

sys.path.insert(0, "/opt/trn_rl_repo")

from contextlib import ExitStack

import numpy as np

import concourse.bass as bass
import concourse.tile as tile
from concourse import mybir
from concourse.bass_utils import run_bass_kernel_spmd
from concourse.vector_clock import ScopedClock, VectorClock

N_CORES = 8
EPS = 1e-5
P = 128


# ---------------------------------------------------------------------------
# Workaround: this walrus build rejects CTRL instructions (Drain/NoOp) with
# more than one sync wait. Tile's final drain carries one wait per live
# processor. Split them across single-wait SP nops; SP program order makes
# this equivalent.
def _patched_drain_and_barrier(self, tick_clock, wait_clock):
    gc = tick_clock.global_clock
    for scope, vclock in ScopedClock({None: gc}).items():
        n = len(vclock)
        for i in range(n):
            if vclock[i] > 0:
                vec = [0] * n
                vec[i] = vclock[i]
                nop_inst = self.nc.sync.nop(nofuse=True, hint="split_drain_wait")
                wait_clock.add_sem_waits(
                    nop_inst.ins, ScopedClock({scope: VectorClock(vec)})
                )
    self.nc.sync.drain()
    self.nc.all_engine_barrier()
    assert self.sems is not None
    popped = self.nc._tile_sem_poison_stack.pop()
    assert popped is self._sem_poison
    self.nc.clear_and_free_semaphores(list(self.sems.allocated().values()))
    self.nc.all_engine_barrier()


tile.TileContext._drain_and_barrier = _patched_drain_and_barrier


# This walrus build allows at most ONE sync wait on ANY instruction. Tile's
# wait-assignment emits up to 4. Post-process the serialized BIR: move all but
# the last wait of each instruction onto same-engine NoOps placed just before
# it (engine program order preserves semantics; for DMAs this gates descriptor
# submission, which is strictly more conservative).
def _split_multi_waits(m: dict) -> dict:
    for fn in m["functions"]:
        for bb in fn["blocks"]:
            out = []
            for ins in bb["instructions"]:
                si = ins.get("sync_info") or {}
                waits = si.get("on_wait") or []
                if len(waits) > 1:
                    for i, w in enumerate(waits[:-1]):
                        out.append(
                            {
                                "debug": ins.get("debug", 0),
                                "engine": ins["engine"],
                                "ins": [],
                                "outs": [],
                                "name": f"{ins['name']}-w{i}",
                                "opcode": "NoOp",
                                "sync_info": {"on_update": [], "on_wait": [w]},
                                "text_hint": "split_wait",
                            }
                        )
                    si["on_wait"] = [waits[-1]]
                out.append(ins)
            bb["instructions"] = out
    return m


_orig_to_json_bytes = bass.Bass.to_json_bytes


def _patched_to_json_bytes(self):
    import orjson

    m = orjson.loads(_orig_to_json_bytes(self))
    return orjson.dumps(_split_multi_waits(m))


bass.Bass.to_json_bytes = _patched_to_json_bytes
# ---------------------------------------------------------------------------


def build_bitlinear_program(b_c, d_in, d_out, apply_invgamma=True, use_fp8=True):
    """Bass program for one core: b_c batch rows, full d_in/d_out.

    use_fp8: run the sign matmul in fp8e4 with DoubleRow (2 MACs/cell/cycle).
    +-1 is exact in fp8e4, so this is lossless; only valid when invgamma is
    not folded into the signs (gamma == 1), else bf16 is used.
    """
    KT = d_in // P  # contraction tiles
    OG = d_out // P  # output-feature tiles
    BT = b_c // P  # batch tiles (stats stage)
    NB = 512  # matmul moving free dim = one PSUM bank of fp32
    BC = b_c // NB  # batch chunks in the matmul/epilogue
    SC = min(512, d_in)  # bn_stats hardware max free size
    nstat = d_in // SC
    if use_fp8:
        assert not apply_invgamma and KT % 2 == 0

    f32 = mybir.dt.float32
    bf16 = mybir.dt.bfloat16
    fp8 = mybir.dt.float8e4
    sdt = fp8 if use_fp8 else bf16  # sign dtype
    wdt = bf16  # weight transfer dtype (sign-preserving host cast)
    X = mybir.AxisListType.X
    A = mybir.AluOpType
    AF = mybir.ActivationFunctionType

    G = min(4, KT)  # k-tiles per transposed-input DMA (8 KiB runs/partition)

    nc = bass.Bass("TRN2", target_bir_lowering=False, debug=False)
    x = nc.dram_tensor("x", [b_c, d_in], f32, kind="ExternalInput")
    # host-prechunked transpose: xTc[h, p, kt, j] = x[h*NB + j, kt*128 + p]
    xTc = nc.dram_tensor("xTc", [BC, P, KT, NB], f32, kind="ExternalInput")
    # host-pretiled weight signsource: w4[og, p, kt, oc] = W[og*128+oc, kt*128+p]
    # so each per-og DMA is one contiguous run per partition. Transferred as
    # bf16 (round-to-nearest preserves the sign of every nonzero float).
    w4 = nc.dram_tensor("w4", [OG, P, KT, P], wdt, kind="ExternalInput")
    bias = nc.dram_tensor("bias", [d_out], f32, kind="ExternalInput")
    beta = nc.dram_tensor("beta", [d_out], f32, kind="ExternalInput")
    gamma = nc.dram_tensor("gamma", [d_in], f32, kind="ExternalInput")
    outT = nc.dram_tensor("outT", [d_out, b_c], f32, kind="ExternalOutput")
    # per-batch-chunk scratches so a chunk's broadcast only depends on that
    # chunk's stats tiles (Tile tracks DRAM deps per tensor).
    mean_ds = [nc.dram_tensor(f"mean_d{h}", [NB], f32) for h in range(BC)]
    c_ds = [nc.dram_tensor(f"c_d{h}", [NB], f32) for h in range(BC)]

    with tile.TileContext(nc) as tc, ExitStack() as ctx:
        consts = ctx.enter_context(tc.tile_pool(name="consts", bufs=1))
        stats_p = ctx.enter_context(tc.tile_pool(name="stats", bufs=NB // P))
        small_p = ctx.enter_context(tc.tile_pool(name="small", bufs=4))
        a_p = ctx.enter_context(tc.tile_pool(name="a", bufs=1))
        xt_p = ctx.enter_context(tc.tile_pool(name="xt", bufs=2))
        w_p = ctx.enter_context(tc.tile_pool(name="w", bufs=2))
        sw_p = ctx.enter_context(tc.tile_pool(name="sw", bufs=5))
        ep_p = ctx.enter_context(tc.tile_pool(name="ep", bufs=4))
        ps_p = ctx.enter_context(tc.tile_pool(name="ps", bufs=2 * BC, space="PSUM"))

        # --- constants ---------------------------------------------------
        eps_t = consts.tile([P, 1], f32)
        nc.vector.memset(eps_t, EPS)
        # column j of these holds v[j*128 : (j+1)*128] (per-partition scalars)
        bias_t = consts.tile([P, OG], f32)
        nc.sync.dma_start(
            out=bias_t, in_=bass.AP(tensor=bias, offset=0, ap=[[1, P], [P, OG]])
        )
        beta_t = consts.tile([P, OG], f32)
        nc.sync.dma_start(
            out=beta_t, in_=bass.AP(tensor=beta, offset=0, ap=[[1, P], [P, OG]])
        )
        bb_t = consts.tile([P, OG], f32)
        nc.vector.tensor_mul(bb_t, bias_t, beta_t)
        if apply_invgamma:
            gamma_t = consts.tile([P, KT], f32)
            nc.sync.dma_start(
                out=gamma_t, in_=bass.AP(tensor=gamma, offset=0, ap=[[1, P], [P, KT]])
            )
            invg = consts.tile([P, KT], f32)
            nc.vector.reciprocal(invg, gamma_t)

        # --- prologue: issue chunk-0's input loads before anything else so
        # they own the DMA queue heads (HBM is ~360 GB/s/core and the early
        # window is oversubscribed), then sign the first weight columns while
        # those loads stream, so the PE has work the moment input signs land.
        TPC0 = NB // P
        QS = d_in // 4
        x_nat0 = []
        for bth in range(TPC0):
            x_nat = stats_p.tile([P, d_in], f32, tag="xnat", name=f"xn{bth}")
            for q in range(4):
                nc.sync.dma_start(
                    out=x_nat[:, q * QS : (q + 1) * QS],
                    in_=x[bth * P : (bth + 1) * P, q * QS : (q + 1) * QS],
                )
            x_nat0.append(x_nat)

        PREW = min(4, OG)
        pre_sw = {}
        for og in range(PREW):
            wcol = w_p.tile([P, KT * P], wdt, tag="wcol")
            nc.sync.dma_start(
                out=wcol,
                in_=bass.AP(
                    tensor=w4, offset=og * P * KT * P, ap=[[KT * P, P], [1, KT * P]]
                ),
            )
            sw = sw_p.tile([P, KT, P], sdt, tag="sw", name=f"swpre{og}")
            nc.scalar.sign(out=sw, in_=wcol.rearrange("p (kt oc) -> p kt oc", oc=P))
            pre_sw[og] = sw

        # --- stages 1-3, chunked by batch chunk (NB rows) so the first
        # chunk's signs (and therefore the matmuls) start after only that
        # chunk's stats. The mean is computed on ScalarE via activation
        # accumulate (fp32 accumulator) so the DVE never sits on the
        # mean->sign critical path; max/min/var work is deferred to after the
        # chunk's signs (it only feeds the epilogue scale c).
        a_t = a_p.tile([P, KT, b_c], sdt)
        junk = consts.tile([P, d_in], bf16)  # ACT accumulate writes land here
        dsc = consts.tile([P, d_in], f32)  # centered-x scratch (stats)
        mean_bs = []
        cbs = []
        TPC = NB // P  # stats tiles per chunk
        inv_n = 1.0 / d_in
        for h in range(BC):
            x_nats = []
            means = []
            # fast path to the chunk's mean: ACT sum + tiny scale
            for bth in range(TPC):
                bt = h * TPC + bth
                if h == 0:
                    x_nat = x_nat0[bth]  # loaded in the prologue
                else:
                    x_nat = stats_p.tile([P, d_in], f32, tag="xnat", name=f"xn{bth}")
                    for q in range(4):
                        nc.sync.dma_start(
                            out=x_nat[:, q * QS : (q + 1) * QS],
                            in_=x[bt * P : (bt + 1) * P, q * QS : (q + 1) * QS],
                        )
                x_nats.append(x_nat)
                # mean+var in one DVE pass chain; ScalarE stays free for the
                # weight-column signs that feed the PE.
                xr = x_nat.rearrange("p (n f) -> p n f", f=SC)
                st = small_p.tile([P, nstat, 6], f32, tag="bnst")
                for i in range(nstat):
                    nc.vector.bn_stats(out=st[:, i, :], in_=xr[:, i, :])
                mv = small_p.tile([P, 2], f32, tag="mv", name=f"mv{bth}")
                nc.vector.bn_aggr(out=mv, in_=st)
                mean = mv[:, 0:1]
                means.append(mv)
                nc.sync.dma_start(out=mean_ds[h][bth * P : (bth + 1) * P], in_=mean)

            # broadcast this chunk's mean across partitions
            mean_b = consts.tile([P, NB], f32, name=f"mean_b{h}")
            nc.sync.dma_start(
                out=mean_b,
                in_=bass.AP(tensor=mean_ds[h], offset=0, ap=[[0, P], [1, NB]]),
            )
            mean_bs.append(mean_b)

            # signs for this chunk, all k tiles (loaded G k-tiles per DMA)
            for gi in range(KT // G):
                xtg = xt_p.tile([P, G, NB], f32, tag="xtg")
                nc.sync.dma_start(
                    out=xtg,
                    in_=bass.AP(
                        tensor=xTc,
                        offset=h * P * KT * NB + gi * G * NB,
                        ap=[[KT * NB, P], [1, G * NB]],
                    ),
                )
                for r in range(G):
                    kt = gi * G + r
                    nc.vector.tensor_sub(xtg[:, r, :], xtg[:, r, :], mean_b)
                    dst = a_t[:, kt, h * NB : (h + 1) * NB]
                    if apply_invgamma:
                        stmp = xt_p.tile([P, NB], bf16, tag="stmp")
                        nc.scalar.sign(out=stmp, in_=xtg[:, r, :])
                        nc.vector.tensor_scalar_mul(
                            out=dst, in0=stmp, scalar1=invg[:, kt : kt + 1]
                        )
                    else:
                        nc.scalar.sign(out=dst, in_=xtg[:, r, :])

            # deferred: c = max|x - mean| / sqrt(var + eps), one centered pass
            for bth in range(TPC):
                x_nat = x_nats[bth]
                mv = means[bth]
                mean = mv[:, 0:1]
                var = mv[:, 1:2]
                nc.vector.tensor_scalar(
                    out=dsc,
                    in0=x_nat,
                    scalar1=mean,
                    scalar2=None,
                    op0=A.subtract,
                )
                amax = small_p.tile([P, 1], f32, tag="amax")
                nc.vector.tensor_reduce(
                    out=amax, in_=dsc, axis=X, op=A.max, apply_absolute_value=True
                )
                std = small_p.tile([P, 1], f32, tag="std")
                nc.scalar.activation(out=std, in_=var, func=AF.Sqrt, bias=eps_t)
                rstd = small_p.tile([P, 1], f32, tag="rstd")
                nc.vector.reciprocal(rstd, std)
                cv = small_p.tile([P, 1], f32, tag="cv")
                nc.vector.tensor_mul(cv, amax, rstd)
                nc.sync.dma_start(out=c_ds[h][bth * P : (bth + 1) * P], in_=cv)

            cb = consts.tile([P, NB], f32, name=f"cb{h}")
            nc.sync.dma_start(
                out=cb, in_=bass.AP(tensor=c_ds[h], offset=0, ap=[[0, P], [1, NB]])
            )
            cbs.append(cb)

        # --- stage 4: weight signs + matmul + epilogue --------------------
        for og in range(OG):
            if og in pre_sw:
                sw = pre_sw[og]
            else:
                wcol = w_p.tile([P, KT * P], wdt, tag="wcol")
                nc.sync.dma_start(
                    out=wcol,
                    in_=bass.AP(
                        tensor=w4,
                        offset=og * P * KT * P,
                        ap=[[KT * P, P], [1, KT * P]],
                    ),
                )
                wcol3 = wcol.rearrange("p (kt oc) -> p kt oc", oc=P)
                sw = sw_p.tile([P, KT, P], sdt, tag="sw")
                nc.scalar.sign(out=sw, in_=wcol3)
            psums = [
                ps_p.tile([P, NB], f32, tag=f"ps{bc}", name=f"psum{bc}")
                for bc in range(BC)
            ]
            if use_fp8:
                for g in range(KT // 2):
                    for bc in range(BC):
                        nc.tensor.matmul(
                            psums[bc],
                            lhsT=sw[:, 2 * g : 2 * g + 2, :],
                            rhs=a_t[:, 2 * g : 2 * g + 2, bc * NB : (bc + 1) * NB],
                            start=(g == 0),
                            stop=(g == KT // 2 - 1),
                            perf_mode=mybir.MatmulPerfMode.DoubleRow,
                        )
            else:
                for kt in range(KT):
                    for bc in range(BC):
                        nc.tensor.matmul(
                            psums[bc],
                            lhsT=sw[:, kt, :],
                            rhs=a_t[:, kt, bc * NB : (bc + 1) * NB],
                            start=(kt == 0),
                            stop=(kt == KT - 1),
                        )
            for bc in range(BC):
                t1 = ep_p.tile([P, NB], f32, tag="t1")
                nc.vector.tensor_tensor(
                    out=t1, in0=psums[bc], in1=cbs[bc], op=A.mult
                )
                # out = t1*beta + bias*beta, per-partition scalars, on ACT
                o_sb = ep_p.tile([P, NB], f32, tag="osb")
                nc.scalar.activation(
                    out=o_sb,
                    in_=t1,
                    func=AF.Identity,
                    bias=bb_t[:, og : og + 1],
                    scale=beta_t[:, og : og + 1],
                )
                nc.sync.dma_start(
                    out=outT[og * P : (og + 1) * P, bc * NB : (bc + 1) * NB],
                    in_=o_sb,
                )

    return nc


def kernel(input, weight, bias, gamma, beta, _run_kwargs=None):
    input = np.ascontiguousarray(np.asarray(input, dtype=np.float32))
    weight = np.ascontiguousarray(np.asarray(weight, dtype=np.float32))
    bias = np.ascontiguousarray(np.asarray(bias, dtype=np.float32))
    gamma = np.ascontiguousarray(np.asarray(gamma, dtype=np.float32))
    beta = np.ascontiguousarray(np.asarray(beta, dtype=np.float32))

    B, d_in = input.shape
    d_out = weight.shape[0]
    assert B % N_CORES == 0
    b_c = B // N_CORES

    apply_invgamma = not bool(np.all(gamma == 1.0))
    use_fp8 = not apply_invgamma
    nc = build_bitlinear_program(
        b_c, d_in, d_out, apply_invgamma=apply_invgamma, use_fp8=use_fp8
    )

    # w4[og, p, kt, oc] = weight[og*128+oc, kt*128+p] — layout transform so
    # each per-og weight DMA is one contiguous run per partition. Shipped as
    # bf16: round-to-nearest preserves the sign of every nonzero float, and
    # the device only takes sign(w).
    import ml_dtypes

    OG, KT = d_out // 128, d_in // 128
    w4 = np.ascontiguousarray(
        weight.reshape(OG, 128, KT, 128).transpose(0, 3, 2, 1)
    ).astype(ml_dtypes.bfloat16)

    NB = 512
    BC = b_c // NB
    in_maps = []
    for c in range(N_CORES):
        sl = slice(c * b_c, (c + 1) * b_c)
        x_c = np.ascontiguousarray(input[sl, :])
        # xTc[h, p, kt, j] = x_c[h*NB + j, kt*128 + p]
        xTc = np.ascontiguousarray(
            x_c.reshape(BC, NB, KT, 128).transpose(0, 3, 2, 1)
        )
        in_maps.append(
            {
                "x": x_c,
                "xTc": xTc,
                "w4": w4,
                "bias": bias,
                "beta": beta,
                "gamma": gamma,
            }
        )

    res = run_bass_kernel_spmd(
        nc, in_maps, core_ids=list(range(N_CORES)), **(_run_kwargs or {})
    )

    out = np.empty((B, d_out), dtype=np.float32)
    for c in range(N_CORES):
        out[c * b_c : (c + 1) * b_c, :] = res.results[c]["outT"].T
    if _run_kwargs:
        kernel.last_results = res
    return out
